# revision 53
# baseline (speedup 1.0000x reference)
"""Fused attention kernel for TRN2, SPMD across 8 NeuronCores.

Problem: out = softmax(mask ? (Q Wq^T + bq)(K Wk^T + bk)^T / sqrt(D) : -1e9)
               @ (V Wv^T + bv)
with B=4, L=2048, E=D=1024.

Sharding: core c handles batch b=c//2, query-half h=c%2 (1024 query rows).
No collectives needed; K/V rows for the batch are fully loaded per core.

Algebra (per core; Xq = Q-shard (1024,E), Xk = K[b] (2048,E), Xv = V[b]):
  scores = (Xq @ Wqk) @ Xk^T + 1 (x) w^T          Wqk = Wq^T Wk
                                                  w   = Xk @ (Wk^T bq)
  (q.bk and bq.bk terms are per-query-row constants and cancel in softmax;
  the 1/sqrt(D) scale is applied at the Exp activation, keeping tT in fp8
  normal range)
  p   = exp(s/32) * mask        (unnormalized; softmax denom deferred)
  out = ((p @ Xv) @ Wv^T) * (1/sum(p)) + 1 (x) bv

Phase dtypes: phase 1 (Q proj) bf16; phase 2 (scores) fp8 e4m3 with
DoubleRow perf mode; phases 4/5 bf16.

Scheduling notes:
 - ALL loads are issued on the SP (sync) queue in consumption order, in
   large consolidated transfers; ACT/DVE streams stay pure compute (a
   dma_start blocks the issuing engine's stream on HWDGE for ~1us).
   Output stores are issued on SP too (it is idle once loads are issued).
 - Phase 1 streams over e1-chunk pairs with a dedicated 8-bank PSUM pool
   (one [128,512] accumulation group per e2t), so the first matmuls only
   need the first wqk/xqT chunk pair (~3.5us) instead of the full 4MB.
 - Software pipeline: scores+softmax of pair k+1 are emitted before the
   AV/out-proj (back) of pair k, so the ACT exp + DVE mask/denom chain of
   k+1 hides under back(k)'s ~20us of PE work.
"""
from contextlib import ExitStack

import numpy as np

import concourse.bacc as bacc
import concourse.tile as tile
from concourse import mybir
from concourse.bass_utils import run_bass_kernel_spmd
from concourse.masks import make_identity

F32 = mybir.dt.float32
BF16 = mybir.dt.bfloat16
FP8 = mybir.dt.float8e4
AF = mybir.ActivationFunctionType
ALU = mybir.AluOpType
DR = mybir.MatmulPerfMode.DoubleRow

B, L, E, D = 4, 2048, 1024, 1024
LS = 1024          # query rows per core
J = 2048           # key rows per core
P = 128
NCORES = 8
SCALE = 1.0 / 32.0  # 1/sqrt(D), applied at the Exp activation

EC = E // P        # 8 chunks of 128 along E/D dims
JC = J // P        # 16 chunks along J
LT = LS // P       # 8 query tiles per core
NP = LT // 2       # 4 query-tile pairs


def _transpose_chunks(nc, ps_tr, src, dst_fn, nblk, ident, psdt, lbl,
                      dve_every=4):
    """Transpose nblk [P,P] blocks of src (groups of 4 share a psum bank).

    src: AP [P, nblk*P]; dst_fn(i) -> destination AP [P, P] for block i.
    1 in dve_every evictions go to DVE, the rest to ACT.
    """
    for t0 in range(0, nblk, 4):
        ps = ps_tr.tile([P, 512], psdt, name=f"pstr_{lbl}", tag="tr")
        for k in range(4):
            nc.tensor.transpose(
                ps[:, k * P:(k + 1) * P],
                src[:, (t0 + k) * P:(t0 + k + 1) * P],
                ident[:],
            )
        for k in range(4):
            dst = dst_fn(t0 + k)
            srcp = ps[:, k * P:(k + 1) * P]
            if (t0 // 4 + k) % dve_every == 0:
                nc.vector.tensor_copy(dst, srcp)
            else:
                nc.scalar.activation(out=dst, in_=srcp, func=AF.Copy)


def _build():
    nc = bacc.Bacc(None, target_bir_lowering=False)

    Xq_e = nc.declare_dram_parameter("XqT", [E, LS], BF16, isOutput=False)
    Xk_e = nc.declare_dram_parameter("XkT", [E, J], FP8, isOutput=False)
    VbH_e = nc.declare_dram_parameter("VbH", [J, E], FP8, isOutput=False)
    VbL_e = nc.declare_dram_parameter("VbL", [J, E], FP8, isOutput=False)
    Mk_e = nc.declare_dram_parameter("mask", [LS, J], FP8, isOutput=False)
    Wqk_e = nc.declare_dram_parameter("Wqk", [E, E], BF16, isOutput=False)
    kb_e = nc.declare_dram_parameter("kb", [E], F32, isOutput=False)
    WvH_e = nc.declare_dram_parameter("WvH", [E, D], FP8, isOutput=False)
    WvL_e = nc.declare_dram_parameter("WvL", [E, D], FP8, isOutput=False)
    bv_e = nc.declare_dram_parameter("bv", [D], F32, isOutput=False)
    out_e = nc.declare_dram_parameter("out", [LS, D], F32, isOutput=True)

    # chunked DRAM views: [p, chunk, free]
    XqT_d = Xq_e.ap().rearrange("(c p) l -> p c l", p=P)
    XkT_d = Xk_e.ap().rearrange("(c p) j -> p c j", p=P)
    VbH_d = VbH_e.ap().rearrange("(c p) e -> p c e", p=P)
    VbL_d = VbL_e.ap().rearrange("(c p) e -> p c e", p=P)
    Wqk_d = Wqk_e.ap().rearrange("(c p) e -> p c e", p=P)
    kb_d = kb_e.ap().rearrange("(c p) -> p c", p=P)
    WvH_d = WvH_e.ap().rearrange("(c p) d -> p c d", p=P)
    WvL_d = WvL_e.ap().rearrange("(c p) d -> p c d", p=P)
    Mk_d = Mk_e.ap().rearrange("(c p) j -> p c j", p=P)
    out_d = out_e.ap().rearrange("(c p) d -> p c d", p=P)

    with tile.TileContext(nc) as tc, ExitStack() as long_pools:
        lp_pool = lambda name: long_pools.enter_context(
            tc.tile_pool(name=name, bufs=1))
        # ---- constants ----
        consts = lp_pool("consts")
        ident_f = consts.tile([P, P], F32, name="ident_f")
        make_identity(nc, ident_f[:])
        ident_b = consts.tile([P, P], BF16, name="ident_b")
        nc.vector.tensor_copy(ident_b[:], ident_f[:])

        bvb_sb = consts.tile([P, D], F32, name="bvb_sb")
        kb_sb = consts.tile([P, EC], F32, name="kb_sb")

        # tT split by l-half so scores for l<512 only depend on the lc0 pass
        tT_lc = [lp_pool(f"tT_p{lc}").tile([P, EC, 512], FP8,
                                           name=f"tT_sb{lc}")
                 for lc in range(2)]
        XkT_sb = lp_pool("XkT_p").tile([P, EC, J], FP8, name="XkT_sb")
        mask_sb = lp_pool("mask_p").tile([P, LT, J], FP8, name="mask_sb")
        WvH_sb = lp_pool("WvH_p").tile([P, EC, D], FP8, name="WvH_sb")
        WvL_sb = lp_pool("WvL_p").tile([P, EC, D], FP8, name="WvL_sb")
        VbH_sb = lp_pool("VbH_p").tile([P, JC, D], FP8, name="VbH_sb")
        VbL_sb = lp_pool("VbL_p").tile([P, JC, D], FP8, name="VbL_sb")

        ppool = lp_pool("pp")
        pmpool = lp_pool("pmp")
        ptpool = lp_pool("ptp")
        dnp = lp_pool("dn")
        ztpool = lp_pool("ztp")
        opool = lp_pool("op")

        def emit_scores(lt, ps_fn):
            # phase 2 (fp8 DoubleRow) + exp -> p_sb bf16 [P, J]
            p_sb = ppool.tile([P, J], BF16, name="p_sb", tag="p",
                              bufs=4)
            tT = tT_lc[lt // 4]
            l0 = (lt % 4) * P
            for jt4 in range(4):
                ps = ps_fn(jt4)
                for e2p in range(EC // 2):
                    nc.tensor.matmul(
                        ps[:],
                        tT[:, 2 * e2p:2 * e2p + 2, l0:l0 + P],
                        XkT_sb[:, 2 * e2p:2 * e2p + 2,
                               jt4 * 512:(jt4 + 1) * 512],
                        start=(e2p == 0), stop=(e2p == EC // 2 - 1),
                        perf_mode=DR,
                    )
                nc.scalar.activation(
                    out=p_sb[:, jt4 * 512:(jt4 + 1) * 512],
                    in_=ps[:], func=AF.Exp, scale=SCALE,
                )
            return p_sb

        def alloc_pT():
            # per-lh tiles so wide (contiguous [P,512]) evictions work
            return {(hl, lh): ptpool.tile([P, JC, P], FP8,
                                          name=f"pT_{hl}{lh}",
                                          tag=f"pt{hl}{lh}", bufs=2)
                    for hl in ("h", "l") for lh in range(2)}

        def emit_tr_group(front, pT, gi):
            # transpose 4 [P,P] blocks of pm and evict hi/lo fp8 with one
            # wide op per engine (ACT: hi, DVE: lo = psum - hi)
            lh, t0 = gi // 4, (gi % 4) * 4
            pm = front[lh][0]
            ps = ps_tr.tile([P, 512], BF16, name="pstr", tag="tr")
            for k in range(4):
                nc.tensor.transpose(
                    ps[:, k * P:(k + 1) * P],
                    pm[:, (t0 + k) * P:(t0 + k + 1) * P],
                    ident_b[:],
                )
            sl = slice(t0 * P, (t0 + 4) * P)
            hi = pT[("h", lh)][:].rearrange("p c l -> p (c l)")[:, sl]
            lo = pT[("l", lh)][:].rearrange("p c l -> p (c l)")[:, sl]
            nc.scalar.activation(out=hi, in_=ps[:], func=AF.Copy)
            nc.vector.tensor_tensor(out=lo, in0=ps[:], in1=hi,
                                    op=ALU.subtract)

        def emit_soft(lt, p_sb):
            # pm = p * mask (unnormalized), accumulate denom; the final
            # per-row scalar is 1/(4*denom): z carries 1/16, WvT carries
            # x64 -> net x4 to cancel
            denom = dnp.tile([P, 1], F32, name="denom", tag="dn",
                             bufs=4)
            pm = pmpool.tile([P, J], BF16, name="pm", tag="pm", bufs=4)
            nc.vector.scalar_tensor_tensor(
                out=pm[:], in0=p_sb[:], scalar=1.0,
                in1=mask_sb[:, lt, :],
                op0=ALU.mult, op1=ALU.mult, accum_out=denom[:],
            )
            den4 = dnp.tile([P, 1], F32, name="den4", tag="d4", bufs=4)
            nc.vector.tensor_scalar(out=den4[:], in0=denom[:],
                                    scalar1=4.0, scalar2=None,
                                    op0=ALU.mult)
            rden = dnp.tile([P, 1], F32, name="rden", tag="rd",
                            bufs=4)
            nc.vector.reciprocal(out=rden[:], in_=den4[:])
            return pm, rden

        # ======== stage A: loads (SP queue) + phase 1 (own psum pool) =====
        ps_tr = long_pools.enter_context(
            tc.tile_pool(name="ps_tr", bufs=3, space="PSUM"))
        with (
            tc.tile_pool(name="ps_p1", bufs=1, space="PSUM") as ps_p1,
            tc.tile_pool(name="wqk_pool", bufs=1) as wqk_pool,
            tc.tile_pool(name="xqt_pool", bufs=1) as xqt_pool,
        ):
            wqk_sb = wqk_pool.tile([P, EC, E], BF16, name="wqk_sb")
            xqT_sb = xqt_pool.tile([P, EC, LS], BF16, name="xqT_sb")
            nc.sync.dma_start(out=kb_sb[:], in_=kb_d)
            # wqk/xqT per chunk, interleaved, in consumption order (chunk
            # arrival ~728ns matches the 4-matmul consumption ~850ns)
            for c in range(EC):
                nc.sync.dma_start(out=wqk_sb[:, c:c + 1, :],
                                  in_=Wqk_d[:, c:c + 1, :])
                nc.sync.dma_start(out=xqT_sb[:, c:c + 1, :],
                                  in_=XqT_d[:, c:c + 1, :])
            # remaining loads, in consumption order
            nc.sync.dma_start(out=XkT_sb[:, 0:4, :], in_=XkT_d[:, 0:4, :])
            nc.sync.dma_start(out=XkT_sb[:, 4:8, :], in_=XkT_d[:, 4:8, :])
            nc.sync.dma_start(out=mask_sb[:, 0:2, :], in_=Mk_d[:, 0:2, :])
            nc.sync.dma_start(out=VbH_sb[:, :, :], in_=VbH_d[:, :, :])
            nc.sync.dma_start(out=VbL_sb[:, :, :], in_=VbL_d[:, :, :])
            nc.sync.dma_start(out=WvH_sb[:, :, :], in_=WvH_d[:, :, :])
            nc.sync.dma_start(out=WvL_sb[:, :, :], in_=WvL_d[:, :, :])
            import concourse.bass as _bass
            bv_bcast = _bass.AP(tensor=bv_e, offset=0,
                                ap=[[0, P], [1, D]])
            nc.sync.dma_start(out=bvb_sb[:], in_=bv_bcast)
            nc.sync.dma_start(out=mask_sb[:, 2:4, :], in_=Mk_d[:, 2:4, :])
            nc.sync.dma_start(out=mask_sb[:, 4:6, :], in_=Mk_d[:, 4:6, :])
            nc.sync.dma_start(out=mask_sb[:, 6:8, :], in_=Mk_d[:, 6:8, :])

            # PE warmup out of the phase-1 psum banks (f32 idents, WAW with
            # the phase-1 groups only orders them on the in-order PE)
            for wu in range(4):
                ps = ps_p1.tile([P, 512], F32, name="pswu",
                                tag=f"p1_{wu % 2}")
                for k in range(4):
                    nc.tensor.transpose(ps[:, k * P:(k + 1) * P],
                                        ident_f[:], ident_f[:])

            def wu_fill(ci):
                # keep the PE warm while streamed chunks arrive (~600ns
                # DMA-paced bubble per chunk in the first lc0 sub-pass)
                for t in range(3):
                    ps = ps_tr.tile([P, 512], BF16, name="pswu2", tag="tr")
                    for k in range(4):
                        nc.tensor.transpose(ps[:, k * P:(k + 1) * P],
                                            ident_b[:], ident_b[:])

            def emit_p1_sub(lc, eh, fill=None):
                # phase 1, e1-chunk streaming over half the e2t groups
                # (4 psum banks); evictions (ACT/DVE alternating) at the
                # sub-pass end. fill(ci): extra PE work per chunk step.
                e2ts = list(range(4 * eh, 4 * eh + 4))
                pss = {e2t: ps_p1.tile([P, 512], F32,
                                       name=f"ps1_{lc}_{e2t}",
                                       tag=f"p1_{e2t - 4 * eh}")
                       for e2t in e2ts}
                for ci in range(EC):
                    for e2t in e2ts:
                        nc.tensor.matmul(
                            pss[e2t][:],
                            wqk_sb[:, ci, e2t * P:(e2t + 1) * P],
                            xqT_sb[:, ci, lc * 512:(lc + 1) * 512],
                            start=(ci == 0), stop=(ci == EC - 1),
                        )
                    if fill is not None:
                        fill(ci)
                # lc0 evictions overlap the lc1 matmuls (ACT is idle
                # there); lc1 evictions land at iteration-0's ACT-heavy
                # start, so they go to DVE instead
                for e2t in e2ts:
                    if lc == 0:
                        nc.scalar.activation(
                            out=tT_lc[lc][:, e2t, :],
                            in_=pss[e2t][:], func=AF.Identity,
                            bias=kb_sb[:, e2t:e2t + 1],
                        )
                    else:
                        nc.vector.tensor_scalar(
                            out=tT_lc[lc][:, e2t, :],
                            in0=pss[e2t][:],
                            scalar1=kb_sb[:, e2t:e2t + 1],
                            scalar2=None, op0=ALU.add,
                        )

            emit_p1_sub(0, 0, fill=wu_fill)
            emit_p1_sub(0, 1)
            # pair-0 scores+softmax; psums reuse the phase-1 banks via
            # matching tags (WAW-ordered). Runs during the lc1 passes.
            front0 = []
            for lt in (0, 1):
                p_sb = emit_scores(
                    lt, lambda jt4, lt=lt: ps_p1.tile(
                        [P, 512], F32, name="ps_sc0",
                        tag=f"p1_{jt4}"))
                front0.append(emit_soft(lt, p_sb))
            # lc1 passes carry pair-0's transposes in their chunk steps
            # (pm(0) becomes ready a little into the first pass)
            pT0 = alloc_pT()
            emit_p1_sub(1, 0, fill=lambda ci: (
                emit_tr_group(front0, pT0, ci - 4) if ci >= 4 else None))
            emit_p1_sub(1, 1, fill=lambda ci: (
                emit_tr_group(front0, pT0, 4 + ci - 4) if ci >= 4 else None))

        # ======== main pools (phase-1 psum pool is closed now) ==========
        with (
            tc.tile_pool(name="ps_s", bufs=2, space="PSUM") as ps_s,
            tc.tile_pool(name="ps_mm", bufs=2, space="PSUM") as ps_mm,
        ):
            def emit_scores_tile(lt, jt4, p_sb):
                ps = ps_s.tile([P, 512], F32, name="ps_sc", tag="s",
                               bufs=3)
                tT = tT_lc[lt // 4]
                l0 = (lt % 4) * P
                for e2p in range(EC // 2):
                    nc.tensor.matmul(
                        ps[:],
                        tT[:, 2 * e2p:2 * e2p + 2, l0:l0 + P],
                        XkT_sb[:, 2 * e2p:2 * e2p + 2,
                               jt4 * 512:(jt4 + 1) * 512],
                        start=(e2p == 0), stop=(e2p == EC // 2 - 1),
                        perf_mode=DR,
                    )
                nc.scalar.activation(
                    out=p_sb[:, jt4 * 512:(jt4 + 1) * 512],
                    in_=ps[:], func=AF.Exp, scale=SCALE,
                )

            def alloc_zT():
                zT_hi = ztpool.tile([P, EC, 2 * P], FP8, name="zT_hi",
                                    tag="zth", bufs=2)
                zT_lo = ztpool.tile([P, EC, 2 * P], FP8, name="zT_lo",
                                    tag="ztl", bufs=2)
                return zT_hi, zT_lo

            def emit_p4_group(pT, zT, dt, lhs):
                # phase 4 dt-group: zT[dt] = Xv^T p^T, fp8 3-term
                # (hi.hi + hi.lo + lo.hi); zT evicted as fp8 hi/lo at 1/16.
                # lhs: which l-halves of the pair to compute ([0,1] or one).
                zT_hi, zT_lo = zT
                terms = [("h", VbH_sb), ("h", VbL_sb), ("l", VbH_sb)]
                nmm = 3 * (JC // 2)
                width = P * len(lhs)
                base = P * lhs[0]
                ps = ps_mm.tile([P, 512], F32, name="ps4", tag="mm")
                for li, lh in enumerate(lhs):
                    n = 0
                    for hl, vb in terms:
                        for jp in range(JC // 2):
                            nc.tensor.matmul(
                                ps[:, li * P:(li + 1) * P],
                                vb[:, 2 * jp:2 * jp + 2,
                                   dt * P:(dt + 1) * P],
                                pT[(hl, lh)][:, 2 * jp:2 * jp + 2, :],
                                start=(n == 0), stop=(n == nmm - 1),
                                perf_mode=DR,
                            )
                            n += 1
                hi = zT_hi[:, dt, base:base + width]
                nc.scalar.activation(out=hi, in_=ps[:, 0:width],
                                     func=AF.Copy, scale=1.0 / 16.0)
                nc.vector.scalar_tensor_tensor(
                    out=zT_lo[:, dt, base:base + width],
                    in0=ps[:, 0:width],
                    scalar=1.0 / 16.0, in1=hi,
                    op0=ALU.mult, op1=ALU.subtract,
                )

            def emit_p5_group(lpair, zT, front, lh, c0, c1, o_sb, eng):
                # phase 5 psum group: out[:, c0:c1] = (zT^T WvT)*rden + bv
                zT_hi, zT_lo = zT
                lt = 2 * lpair + lh
                rden = front[lh][1]
                terms = [(zT_hi, WvH_sb), (zT_hi, WvL_sb), (zT_lo, WvH_sb)]
                nmm = 3 * EC // 2
                ps = ps_mm.tile([P, 512], F32, name="ps5", tag="mm")
                n = 0
                for zt, wv in terms:
                    for dp in range(EC // 2):
                        nc.tensor.matmul(
                            ps[:, 0:c1 - c0],
                            zt[:, 2 * dp:2 * dp + 2, lh * P:(lh + 1) * P],
                            wv[:, 2 * dp:2 * dp + 2, c0:c1],
                            start=(n == 0), stop=(n == nmm - 1),
                            perf_mode=DR,
                        )
                        n += 1
                nc.vector.scalar_tensor_tensor(
                    out=o_sb[:, c0:c1],
                    in0=ps[:, 0:c1 - c0], scalar=rden[:],
                    in1=bvb_sb[:, c0:c1],
                    op0=ALU.mult, op1=ALU.add,
                )
                eng.dma_start(out=out_d[:, lt, c0:c1],
                              in_=o_sb[:, c0:c1])

            # ===== main software pipeline =====
            # Iteration k: scores(k+1)+soft(k+1) head, then p4(k) with
            # tr(k+1) groups riding the dt-groups (1.3us of PE work each
            # hides the wide transpose evictions), then p5(k).  Pair-0's
            # tr rode inside the phase-1 lc1 passes.
            front, pT = front0, pT0

            def emit_head(nxt_lts):
                p_sbs = {lt: ppool.tile([P, J], BF16, name="p_sb",
                                        tag="p", bufs=4)
                         for lt in nxt_lts}
                nfront = []
                for lt in nxt_lts:
                    for jt4 in range(4):
                        emit_scores_tile(lt, jt4, p_sbs[lt])
                    nfront.append(emit_soft(lt, p_sbs[lt]))
                return nfront

            for lpair in range(NP):
                last = lpair == NP - 1
                if not last:
                    nfront = emit_head([2 * lpair + 2, 2 * lpair + 3])
                    pTn = alloc_pT()
                    zT = alloc_zT()
                    for dt in range(EC):
                        emit_p4_group(pT, zT, dt, [0, 1])
                        emit_tr_group(nfront, pTn, dt)
                    o_sbs = {lh: opool.tile([P, D], F32, name="o_sb",
                                            tag="o", bufs=3)
                             for lh in range(2)}
                    for lh in range(2):
                        for doc in range(2):
                            emit_p5_group(lpair, zT, front, lh,
                                          doc * 512, (doc + 1) * 512,
                                          o_sbs[lh], nc.sync)
                    front, pT = nfront, pTn
                else:
                    # last pair: tr already done; split p4 by l-half so
                    # the tail drains sooner; the final l-half's stores go
                    # out in 256-wide pieces on alternating DMA queues so
                    # their issue latencies overlap
                    for lh in range(2):
                        zT = alloc_zT()
                        for dt in range(EC):
                            emit_p4_group(pT, zT, dt, [lh])
                        o_sb = opool.tile([P, D], F32, name="o_sb",
                                          tag="o", bufs=3)
                        npc = 2 if lh == 0 else 4
                        w = D // npc
                        for pc in range(npc):
                            eng = nc.sync if pc % 2 == 0 else nc.scalar
                            emit_p5_group(lpair, zT, front, lh,
                                          pc * w, (pc + 1) * w,
                                          o_sb, eng)

    nc.compile()
    return nc


_NC_CACHE = {}


def _get_nc():
    if "nc" not in _NC_CACHE:
        _NC_CACHE["nc"] = _build()
    return _NC_CACHE["nc"]


def _shard_inputs(Q, K, V, mask, Wq_w, Wq_b, Wk_w, Wk_b, Wv_w, Wv_b):
    import ml_dtypes
    bf16 = ml_dtypes.bfloat16
    fp8 = ml_dtypes.float8_e4m3
    f32 = np.float32

    def hilo(x):
        hi = x.astype(fp8)
        lo = (x - hi.astype(f32)).astype(fp8)
        return (np.ascontiguousarray(hi), np.ascontiguousarray(lo))

    Wq32 = np.asarray(Wq_w, f32)
    Wk32 = np.asarray(Wk_w, f32)
    # NOTE: the 1/sqrt(D) score scale is applied at the Exp activation
    # (scale=1/32), so Wqk/kb are unscaled here — keeps tT in fp8's
    # normal range (sigma ~ 0.33). WvT is scaled x64 so its fp8 hi/lo
    # escapes the subnormal floor; z carries 1/16 — both cancelled by
    # the final 1/(4*denom) row scale.
    WvH, WvL = hilo(np.asarray(Wv_w, f32).T * 64.0)
    common = {
        "Wqk": np.ascontiguousarray(
            (Wq32.T @ Wk32).astype(bf16)),
        "kb": np.ascontiguousarray(
            Wk32.T @ np.asarray(Wq_b, f32), f32),
        "WvH": WvH, "WvL": WvL,
        "bv": np.ascontiguousarray(Wv_b, f32),
    }
    in_maps = []
    for c in range(NCORES):
        b, h = divmod(c, 2)
        sl = slice(h * LS, (h + 1) * LS)
        VbH, VbL = hilo(np.asarray(V[b], f32))
        in_maps.append({
            "XqT": np.ascontiguousarray(
                np.asarray(Q[b, sl, :], f32).astype(bf16).T),
            "XkT": np.ascontiguousarray(
                np.asarray(K[b], f32).astype(fp8).T),
            "VbH": VbH, "VbL": VbL,
            "mask": np.ascontiguousarray(
                np.asarray(mask[b, sl, :]).astype(fp8)),
            **common,
        })
    return in_maps


def _run(inputs, trace=False):
    nc = _get_nc()
    in_maps = _shard_inputs(**inputs)
    res = run_bass_kernel_spmd(nc, in_maps, core_ids=list(range(NCORES)),
                               trace=trace)
    out = np.empty((B, L, D), np.float32)
    for c in range(NCORES):
        b, h = divmod(c, 2)
        out[b, h * LS:(h + 1) * LS, :] = res.results[c]["out"]
    return out, res


def kernel(**inputs):
    out, _ = _run(inputs, trace=False)
    return out


# revision 54
# speedup vs baseline: 1.0048x; 1.0048x over previous
"""Fused attention kernel for TRN2, SPMD across 8 NeuronCores.

Problem: out = softmax(mask ? (Q Wq^T + bq)(K Wk^T + bk)^T / sqrt(D) : -1e9)
               @ (V Wv^T + bv)
with B=4, L=2048, E=D=1024.

Sharding: core c handles batch b=c//2, query-half h=c%2 (1024 query rows).
No collectives needed; K/V rows for the batch are fully loaded per core.

Algebra (per core; Xq = Q-shard (1024,E), Xk = K[b] (2048,E), Xv = V[b]):
  scores = (Xq @ Wqk) @ Xk^T + 1 (x) w^T          Wqk = Wq^T Wk
                                                  w   = Xk @ (Wk^T bq)
  (q.bk and bq.bk terms are per-query-row constants and cancel in softmax;
  the 1/sqrt(D) scale is applied at the Exp activation, keeping tT in fp8
  normal range)
  p   = exp(s/32) * mask        (unnormalized; softmax denom deferred)
  out = ((p @ Xv) @ Wv^T) * (1/sum(p)) + 1 (x) bv

Phase dtypes: phase 1 (Q proj) bf16; phase 2 (scores) fp8 e4m3 with
DoubleRow perf mode; phases 4/5 bf16.

Scheduling notes:
 - ALL loads are issued on the SP (sync) queue in consumption order, in
   large consolidated transfers; ACT/DVE streams stay pure compute (a
   dma_start blocks the issuing engine's stream on HWDGE for ~1us).
   Output stores are issued on SP too (it is idle once loads are issued).
 - Phase 1 streams over e1-chunk pairs with a dedicated 8-bank PSUM pool
   (one [128,512] accumulation group per e2t), so the first matmuls only
   need the first wqk/xqT chunk pair (~3.5us) instead of the full 4MB.
 - Software pipeline: scores+softmax of pair k+1 are emitted before the
   AV/out-proj (back) of pair k, so the ACT exp + DVE mask/denom chain of
   k+1 hides under back(k)'s ~20us of PE work.
"""
from contextlib import ExitStack

import numpy as np

import concourse.bacc as bacc
import concourse.tile as tile
from concourse import mybir
from concourse.bass_utils import run_bass_kernel_spmd
from concourse.masks import make_identity

F32 = mybir.dt.float32
BF16 = mybir.dt.bfloat16
FP8 = mybir.dt.float8e4
AF = mybir.ActivationFunctionType
ALU = mybir.AluOpType
DR = mybir.MatmulPerfMode.DoubleRow

B, L, E, D = 4, 2048, 1024, 1024
LS = 1024          # query rows per core
J = 2048           # key rows per core
P = 128
NCORES = 8
SCALE = 1.0 / 32.0  # 1/sqrt(D), applied at the Exp activation

EC = E // P        # 8 chunks of 128 along E/D dims
JC = J // P        # 16 chunks along J
LT = LS // P       # 8 query tiles per core
NP = LT // 2       # 4 query-tile pairs


def _transpose_chunks(nc, ps_tr, src, dst_fn, nblk, ident, psdt, lbl,
                      dve_every=4):
    """Transpose nblk [P,P] blocks of src (groups of 4 share a psum bank).

    src: AP [P, nblk*P]; dst_fn(i) -> destination AP [P, P] for block i.
    1 in dve_every evictions go to DVE, the rest to ACT.
    """
    for t0 in range(0, nblk, 4):
        ps = ps_tr.tile([P, 512], psdt, name=f"pstr_{lbl}", tag="tr")
        for k in range(4):
            nc.tensor.transpose(
                ps[:, k * P:(k + 1) * P],
                src[:, (t0 + k) * P:(t0 + k + 1) * P],
                ident[:],
            )
        for k in range(4):
            dst = dst_fn(t0 + k)
            srcp = ps[:, k * P:(k + 1) * P]
            if (t0 // 4 + k) % dve_every == 0:
                nc.vector.tensor_copy(dst, srcp)
            else:
                nc.scalar.activation(out=dst, in_=srcp, func=AF.Copy)


def _build():
    nc = bacc.Bacc(None, target_bir_lowering=False)

    Xq_e = nc.declare_dram_parameter("XqT", [E, LS], BF16, isOutput=False)
    Xk_e = nc.declare_dram_parameter("XkT", [E, J], FP8, isOutput=False)
    VbH_e = nc.declare_dram_parameter("VbH", [J, E], FP8, isOutput=False)
    VbL_e = nc.declare_dram_parameter("VbL", [J, E], FP8, isOutput=False)
    Mk_e = nc.declare_dram_parameter("mask", [LS, J], FP8, isOutput=False)
    Wqk_e = nc.declare_dram_parameter("Wqk", [E, E], BF16, isOutput=False)
    kb_e = nc.declare_dram_parameter("kb", [E], F32, isOutput=False)
    WvH_e = nc.declare_dram_parameter("WvH", [E, D], FP8, isOutput=False)
    WvL_e = nc.declare_dram_parameter("WvL", [E, D], FP8, isOutput=False)
    bv_e = nc.declare_dram_parameter("bv", [D], F32, isOutput=False)
    out_e = nc.declare_dram_parameter("out", [LS, D], F32, isOutput=True)

    # chunked DRAM views: [p, chunk, free]
    XqT_d = Xq_e.ap().rearrange("(c p) l -> p c l", p=P)
    XkT_d = Xk_e.ap().rearrange("(c p) j -> p c j", p=P)
    VbH_d = VbH_e.ap().rearrange("(c p) e -> p c e", p=P)
    VbL_d = VbL_e.ap().rearrange("(c p) e -> p c e", p=P)
    Wqk_d = Wqk_e.ap().rearrange("(c p) e -> p c e", p=P)
    kb_d = kb_e.ap().rearrange("(c p) -> p c", p=P)
    WvH_d = WvH_e.ap().rearrange("(c p) d -> p c d", p=P)
    WvL_d = WvL_e.ap().rearrange("(c p) d -> p c d", p=P)
    Mk_d = Mk_e.ap().rearrange("(c p) j -> p c j", p=P)
    out_d = out_e.ap().rearrange("(c p) d -> p c d", p=P)

    with tile.TileContext(nc) as tc, ExitStack() as long_pools:
        lp_pool = lambda name: long_pools.enter_context(
            tc.tile_pool(name=name, bufs=1))
        # ---- constants ----
        consts = lp_pool("consts")
        ident_f = consts.tile([P, P], F32, name="ident_f")
        make_identity(nc, ident_f[:])
        ident_b = consts.tile([P, P], BF16, name="ident_b")
        nc.vector.tensor_copy(ident_b[:], ident_f[:])

        bvb_sb = consts.tile([P, D], F32, name="bvb_sb")
        kb_sb = consts.tile([P, EC], F32, name="kb_sb")

        # tT split by l-half so scores for l<512 only depend on the lc0 pass
        tT_lc = [lp_pool(f"tT_p{lc}").tile([P, EC, 512], FP8,
                                           name=f"tT_sb{lc}")
                 for lc in range(2)]
        XkT_sb = lp_pool("XkT_p").tile([P, EC, J], FP8, name="XkT_sb")
        mask_sb = lp_pool("mask_p").tile([P, LT, J], FP8, name="mask_sb")
        WvH_sb = lp_pool("WvH_p").tile([P, EC, D], FP8, name="WvH_sb")
        WvL_sb = lp_pool("WvL_p").tile([P, EC, D], FP8, name="WvL_sb")
        VbH_sb = lp_pool("VbH_p").tile([P, JC, D], FP8, name="VbH_sb")
        VbL_sb = lp_pool("VbL_p").tile([P, JC, D], FP8, name="VbL_sb")

        ppool = lp_pool("pp")
        pmpool = lp_pool("pmp")
        ptpool = lp_pool("ptp")
        dnp = lp_pool("dn")
        ztpool = lp_pool("ztp")
        opool = lp_pool("op")

        def emit_scores(lt, ps_fn):
            # phase 2 (fp8 DoubleRow) + exp -> p_sb bf16 [P, J]
            p_sb = ppool.tile([P, J], BF16, name="p_sb", tag="p",
                              bufs=4)
            tT = tT_lc[lt // 4]
            l0 = (lt % 4) * P
            for jt4 in range(4):
                ps = ps_fn(jt4)
                for e2p in range(EC // 2):
                    nc.tensor.matmul(
                        ps[:],
                        tT[:, 2 * e2p:2 * e2p + 2, l0:l0 + P],
                        XkT_sb[:, 2 * e2p:2 * e2p + 2,
                               jt4 * 512:(jt4 + 1) * 512],
                        start=(e2p == 0), stop=(e2p == EC // 2 - 1),
                        perf_mode=DR,
                    )
                nc.scalar.activation(
                    out=p_sb[:, jt4 * 512:(jt4 + 1) * 512],
                    in_=ps[:], func=AF.Exp, scale=SCALE,
                )
            return p_sb

        def alloc_pT():
            # per-lh tiles so wide (contiguous [P,512]) evictions work
            return {(hl, lh): ptpool.tile([P, JC, P], FP8,
                                          name=f"pT_{hl}{lh}",
                                          tag=f"pt{hl}{lh}", bufs=2)
                    for hl in ("h", "l") for lh in range(2)}

        def emit_tr_group(front, pT, gi):
            # transpose 4 [P,P] blocks of pm and evict hi/lo fp8 with one
            # wide op per engine (ACT: hi, DVE: lo = psum - hi)
            lh, t0 = gi // 4, (gi % 4) * 4
            pm = front[lh][0]
            ps = ps_tr.tile([P, 512], BF16, name="pstr", tag="tr")
            for k in range(4):
                nc.tensor.transpose(
                    ps[:, k * P:(k + 1) * P],
                    pm[:, (t0 + k) * P:(t0 + k + 1) * P],
                    ident_b[:],
                )
            sl = slice(t0 * P, (t0 + 4) * P)
            hi = pT[("h", lh)][:].rearrange("p c l -> p (c l)")[:, sl]
            lo = pT[("l", lh)][:].rearrange("p c l -> p (c l)")[:, sl]
            nc.scalar.activation(out=hi, in_=ps[:], func=AF.Copy)
            nc.vector.tensor_tensor(out=lo, in0=ps[:], in1=hi,
                                    op=ALU.subtract)

        def emit_soft(lt, p_sb):
            # pm = p * mask (unnormalized), accumulate denom; the final
            # per-row scalar is 1/(4*denom): z carries 1/16, WvT carries
            # x64 -> net x4 to cancel
            denom = dnp.tile([P, 1], F32, name="denom", tag="dn",
                             bufs=4)
            pm = pmpool.tile([P, J], BF16, name="pm", tag="pm", bufs=4)
            nc.vector.scalar_tensor_tensor(
                out=pm[:], in0=p_sb[:], scalar=1.0,
                in1=mask_sb[:, lt, :],
                op0=ALU.mult, op1=ALU.mult, accum_out=denom[:],
            )
            den4 = dnp.tile([P, 1], F32, name="den4", tag="d4", bufs=4)
            nc.vector.tensor_scalar(out=den4[:], in0=denom[:],
                                    scalar1=4.0, scalar2=None,
                                    op0=ALU.mult)
            rden = dnp.tile([P, 1], F32, name="rden", tag="rd",
                            bufs=4)
            nc.vector.reciprocal(out=rden[:], in_=den4[:])
            return pm, rden

        # ======== stage A: loads (SP queue) + phase 1 (own psum pool) =====
        ps_tr = long_pools.enter_context(
            tc.tile_pool(name="ps_tr", bufs=3, space="PSUM"))
        with (
            tc.tile_pool(name="ps_p1", bufs=1, space="PSUM") as ps_p1,
            tc.tile_pool(name="wqk_pool", bufs=1) as wqk_pool,
            tc.tile_pool(name="xqt_pool", bufs=1) as xqt_pool,
        ):
            wqk_sb = wqk_pool.tile([P, EC, E], BF16, name="wqk_sb")
            xqT_sb = xqt_pool.tile([P, EC, LS], BF16, name="xqT_sb")
            nc.sync.dma_start(out=kb_sb[:], in_=kb_d)
            # wqk/xqT per chunk, interleaved, in consumption order (chunk
            # arrival ~728ns matches the 4-matmul consumption ~850ns)
            for c in range(EC):
                nc.sync.dma_start(out=wqk_sb[:, c:c + 1, :],
                                  in_=Wqk_d[:, c:c + 1, :])
                nc.sync.dma_start(out=xqT_sb[:, c:c + 1, :],
                                  in_=XqT_d[:, c:c + 1, :])
            # remaining loads, in consumption order
            nc.sync.dma_start(out=XkT_sb[:, 0:4, :], in_=XkT_d[:, 0:4, :])
            nc.sync.dma_start(out=XkT_sb[:, 4:8, :], in_=XkT_d[:, 4:8, :])
            nc.sync.dma_start(out=mask_sb[:, 0:2, :], in_=Mk_d[:, 0:2, :])
            nc.sync.dma_start(out=VbH_sb[:, :, :], in_=VbH_d[:, :, :])
            nc.sync.dma_start(out=VbL_sb[:, :, :], in_=VbL_d[:, :, :])
            nc.sync.dma_start(out=WvH_sb[:, :, :], in_=WvH_d[:, :, :])
            nc.sync.dma_start(out=WvL_sb[:, :, :], in_=WvL_d[:, :, :])
            import concourse.bass as _bass
            bv_bcast = _bass.AP(tensor=bv_e, offset=0,
                                ap=[[0, P], [1, D]])
            nc.sync.dma_start(out=bvb_sb[:], in_=bv_bcast)
            nc.sync.dma_start(out=mask_sb[:, 2:4, :], in_=Mk_d[:, 2:4, :])
            nc.sync.dma_start(out=mask_sb[:, 4:6, :], in_=Mk_d[:, 4:6, :])
            nc.sync.dma_start(out=mask_sb[:, 6:8, :], in_=Mk_d[:, 6:8, :])

            # PE warmup out of the phase-1 psum banks (f32 idents, WAW with
            # the phase-1 groups only orders them on the in-order PE)
            for wu in range(4):
                ps = ps_p1.tile([P, 512], F32, name="pswu",
                                tag=f"p1_{wu % 2}")
                for k in range(4):
                    nc.tensor.transpose(ps[:, k * P:(k + 1) * P],
                                        ident_f[:], ident_f[:])

            def wu_fill(ci):
                # keep the PE warm while streamed chunks arrive (~600ns
                # DMA-paced bubble per chunk in the first lc0 sub-pass)
                for t in range(3):
                    ps = ps_tr.tile([P, 512], BF16, name="pswu2", tag="tr")
                    for k in range(4):
                        nc.tensor.transpose(ps[:, k * P:(k + 1) * P],
                                            ident_b[:], ident_b[:])

            def emit_p1_sub(lc, eh, fill=None):
                # phase 1, e1-chunk streaming over half the e2t groups
                # (4 psum banks); evictions (ACT/DVE alternating) at the
                # sub-pass end. fill(ci): extra PE work per chunk step.
                e2ts = list(range(4 * eh, 4 * eh + 4))
                pss = {e2t: ps_p1.tile([P, 512], F32,
                                       name=f"ps1_{lc}_{e2t}",
                                       tag=f"p1_{e2t - 4 * eh}")
                       for e2t in e2ts}
                for ci in range(EC):
                    for e2t in e2ts:
                        nc.tensor.matmul(
                            pss[e2t][:],
                            wqk_sb[:, ci, e2t * P:(e2t + 1) * P],
                            xqT_sb[:, ci, lc * 512:(lc + 1) * 512],
                            start=(ci == 0), stop=(ci == EC - 1),
                        )
                    if fill is not None:
                        fill(ci)
                # lc0 evictions overlap the lc1 matmuls (ACT is idle
                # there); lc1 evictions land at iteration-0's ACT-heavy
                # start, so they go to DVE instead
                for e2t in e2ts:
                    if lc == 0 and e2t % 2 == 0:
                        nc.scalar.activation(
                            out=tT_lc[lc][:, e2t, :],
                            in_=pss[e2t][:], func=AF.Identity,
                            bias=kb_sb[:, e2t:e2t + 1],
                        )
                    else:
                        nc.vector.tensor_scalar(
                            out=tT_lc[lc][:, e2t, :],
                            in0=pss[e2t][:],
                            scalar1=kb_sb[:, e2t:e2t + 1],
                            scalar2=None, op0=ALU.add,
                        )

            emit_p1_sub(0, 0, fill=wu_fill)
            emit_p1_sub(0, 1)
            # pair-0 scores+softmax; psums reuse the phase-1 banks via
            # matching tags (WAW-ordered). Runs during the lc1 passes.
            front0 = []
            for lt in (0, 1):
                p_sb = emit_scores(
                    lt, lambda jt4, lt=lt: ps_p1.tile(
                        [P, 512], F32, name="ps_sc0",
                        tag=f"p1_{jt4}"))
                front0.append(emit_soft(lt, p_sb))
            # lc1 passes carry pair-0's transposes in their chunk steps
            # (pm(0) becomes ready a little into the first pass)
            pT0 = alloc_pT()
            emit_p1_sub(1, 0, fill=lambda ci: (
                emit_tr_group(front0, pT0, ci - 4) if ci >= 4 else None))
            emit_p1_sub(1, 1, fill=lambda ci: (
                emit_tr_group(front0, pT0, 4 + ci - 4) if ci >= 4 else None))

        # ======== main pools (phase-1 psum pool is closed now) ==========
        with (
            tc.tile_pool(name="ps_s", bufs=2, space="PSUM") as ps_s,
            tc.tile_pool(name="ps_mm", bufs=2, space="PSUM") as ps_mm,
        ):
            def emit_scores_tile(lt, jt4, p_sb):
                ps = ps_s.tile([P, 512], F32, name="ps_sc", tag="s",
                               bufs=3)
                tT = tT_lc[lt // 4]
                l0 = (lt % 4) * P
                for e2p in range(EC // 2):
                    nc.tensor.matmul(
                        ps[:],
                        tT[:, 2 * e2p:2 * e2p + 2, l0:l0 + P],
                        XkT_sb[:, 2 * e2p:2 * e2p + 2,
                               jt4 * 512:(jt4 + 1) * 512],
                        start=(e2p == 0), stop=(e2p == EC // 2 - 1),
                        perf_mode=DR,
                    )
                nc.scalar.activation(
                    out=p_sb[:, jt4 * 512:(jt4 + 1) * 512],
                    in_=ps[:], func=AF.Exp, scale=SCALE,
                )

            def alloc_zT():
                zT_hi = ztpool.tile([P, EC, 2 * P], FP8, name="zT_hi",
                                    tag="zth", bufs=2)
                zT_lo = ztpool.tile([P, EC, 2 * P], FP8, name="zT_lo",
                                    tag="ztl", bufs=2)
                return zT_hi, zT_lo

            def emit_p4_group(pT, zT, dt, lhs):
                # phase 4 dt-group: zT[dt] = Xv^T p^T, fp8 3-term
                # (hi.hi + hi.lo + lo.hi); zT evicted as fp8 hi/lo at 1/16.
                # lhs: which l-halves of the pair to compute ([0,1] or one).
                zT_hi, zT_lo = zT
                terms = [("h", VbH_sb), ("h", VbL_sb), ("l", VbH_sb)]
                nmm = 3 * (JC // 2)
                width = P * len(lhs)
                base = P * lhs[0]
                ps = ps_mm.tile([P, 512], F32, name="ps4", tag="mm")
                for li, lh in enumerate(lhs):
                    n = 0
                    for hl, vb in terms:
                        for jp in range(JC // 2):
                            nc.tensor.matmul(
                                ps[:, li * P:(li + 1) * P],
                                vb[:, 2 * jp:2 * jp + 2,
                                   dt * P:(dt + 1) * P],
                                pT[(hl, lh)][:, 2 * jp:2 * jp + 2, :],
                                start=(n == 0), stop=(n == nmm - 1),
                                perf_mode=DR,
                            )
                            n += 1
                hi = zT_hi[:, dt, base:base + width]
                nc.scalar.activation(out=hi, in_=ps[:, 0:width],
                                     func=AF.Copy, scale=1.0 / 16.0)
                nc.vector.scalar_tensor_tensor(
                    out=zT_lo[:, dt, base:base + width],
                    in0=ps[:, 0:width],
                    scalar=1.0 / 16.0, in1=hi,
                    op0=ALU.mult, op1=ALU.subtract,
                )

            def emit_p5_group(lpair, zT, front, lh, c0, c1, o_sb, eng):
                # phase 5 psum group: out[:, c0:c1] = (zT^T WvT)*rden + bv
                zT_hi, zT_lo = zT
                lt = 2 * lpair + lh
                rden = front[lh][1]
                terms = [(zT_hi, WvH_sb), (zT_hi, WvL_sb), (zT_lo, WvH_sb)]
                nmm = 3 * EC // 2
                ps = ps_mm.tile([P, 512], F32, name="ps5", tag="mm")
                n = 0
                for zt, wv in terms:
                    for dp in range(EC // 2):
                        nc.tensor.matmul(
                            ps[:, 0:c1 - c0],
                            zt[:, 2 * dp:2 * dp + 2, lh * P:(lh + 1) * P],
                            wv[:, 2 * dp:2 * dp + 2, c0:c1],
                            start=(n == 0), stop=(n == nmm - 1),
                            perf_mode=DR,
                        )
                        n += 1
                nc.vector.scalar_tensor_tensor(
                    out=o_sb[:, c0:c1],
                    in0=ps[:, 0:c1 - c0], scalar=rden[:],
                    in1=bvb_sb[:, c0:c1],
                    op0=ALU.mult, op1=ALU.add,
                )
                eng.dma_start(out=out_d[:, lt, c0:c1],
                              in_=o_sb[:, c0:c1])

            # ===== main software pipeline =====
            # Iteration k: scores(k+1)+soft(k+1) head, then p4(k) with
            # tr(k+1) groups riding the dt-groups (1.3us of PE work each
            # hides the wide transpose evictions), then p5(k).  Pair-0's
            # tr rode inside the phase-1 lc1 passes.
            front, pT = front0, pT0

            def emit_head(nxt_lts):
                p_sbs = {lt: ppool.tile([P, J], BF16, name="p_sb",
                                        tag="p", bufs=4)
                         for lt in nxt_lts}
                nfront = []
                for lt in nxt_lts:
                    for jt4 in range(4):
                        emit_scores_tile(lt, jt4, p_sbs[lt])
                    nfront.append(emit_soft(lt, p_sbs[lt]))
                return nfront

            for lpair in range(NP):
                last = lpair == NP - 1
                if not last:
                    nfront = emit_head([2 * lpair + 2, 2 * lpair + 3])
                    pTn = alloc_pT()
                    zT = alloc_zT()
                    for dt in range(EC):
                        emit_p4_group(pT, zT, dt, [0, 1])
                        emit_tr_group(nfront, pTn, dt)
                    o_sbs = {lh: opool.tile([P, D], F32, name="o_sb",
                                            tag="o", bufs=3)
                             for lh in range(2)}
                    for lh in range(2):
                        for doc in range(2):
                            emit_p5_group(lpair, zT, front, lh,
                                          doc * 512, (doc + 1) * 512,
                                          o_sbs[lh], nc.sync)
                    front, pT = nfront, pTn
                else:
                    # last pair: tr already done; split p4 by l-half so
                    # the tail drains sooner; the final l-half's stores go
                    # out in 256-wide pieces on alternating DMA queues so
                    # their issue latencies overlap
                    for lh in range(2):
                        zT = alloc_zT()
                        for dt in range(EC):
                            emit_p4_group(pT, zT, dt, [lh])
                        o_sb = opool.tile([P, D], F32, name="o_sb",
                                          tag="o", bufs=3)
                        npc = 2 if lh == 0 else 4
                        w = D // npc
                        for pc in range(npc):
                            eng = nc.sync if pc % 2 == 0 else nc.scalar
                            emit_p5_group(lpair, zT, front, lh,
                                          pc * w, (pc + 1) * w,
                                          o_sb, eng)

    nc.compile()
    return nc


_NC_CACHE = {}


def _get_nc():
    if "nc" not in _NC_CACHE:
        _NC_CACHE["nc"] = _build()
    return _NC_CACHE["nc"]


def _shard_inputs(Q, K, V, mask, Wq_w, Wq_b, Wk_w, Wk_b, Wv_w, Wv_b):
    import ml_dtypes
    bf16 = ml_dtypes.bfloat16
    fp8 = ml_dtypes.float8_e4m3
    f32 = np.float32

    def hilo(x):
        hi = x.astype(fp8)
        lo = (x - hi.astype(f32)).astype(fp8)
        return (np.ascontiguousarray(hi), np.ascontiguousarray(lo))

    Wq32 = np.asarray(Wq_w, f32)
    Wk32 = np.asarray(Wk_w, f32)
    # NOTE: the 1/sqrt(D) score scale is applied at the Exp activation
    # (scale=1/32), so Wqk/kb are unscaled here — keeps tT in fp8's
    # normal range (sigma ~ 0.33). WvT is scaled x64 so its fp8 hi/lo
    # escapes the subnormal floor; z carries 1/16 — both cancelled by
    # the final 1/(4*denom) row scale.
    WvH, WvL = hilo(np.asarray(Wv_w, f32).T * 64.0)
    common = {
        "Wqk": np.ascontiguousarray(
            (Wq32.T @ Wk32).astype(bf16)),
        "kb": np.ascontiguousarray(
            Wk32.T @ np.asarray(Wq_b, f32), f32),
        "WvH": WvH, "WvL": WvL,
        "bv": np.ascontiguousarray(Wv_b, f32),
    }
    in_maps = []
    for c in range(NCORES):
        b, h = divmod(c, 2)
        sl = slice(h * LS, (h + 1) * LS)
        VbH, VbL = hilo(np.asarray(V[b], f32))
        in_maps.append({
            "XqT": np.ascontiguousarray(
                np.asarray(Q[b, sl, :], f32).astype(bf16).T),
            "XkT": np.ascontiguousarray(
                np.asarray(K[b], f32).astype(fp8).T),
            "VbH": VbH, "VbL": VbL,
            "mask": np.ascontiguousarray(
                np.asarray(mask[b, sl, :]).astype(fp8)),
            **common,
        })
    return in_maps


def _run(inputs, trace=False):
    nc = _get_nc()
    in_maps = _shard_inputs(**inputs)
    res = run_bass_kernel_spmd(nc, in_maps, core_ids=list(range(NCORES)),
                               trace=trace)
    out = np.empty((B, L, D), np.float32)
    for c in range(NCORES):
        b, h = divmod(c, 2)
        out[b, h * LS:(h + 1) * LS, :] = res.results[c]["out"]
    return out, res


def kernel(**inputs):
    out, _ = _run(inputs, trace=False)
    return out
